# revision 1
# baseline (speedup 1.0000x reference)
"""Trainium2 Bass kernel for thresholded multi-head attention.

Computes, for x:[b,n,dim] with b=4, n=2048, dim=512, heads=8, dh=64:
    qkv = x @ Wqkv + bqkv ; split q,k,v per head
    dots = q k^T / sqrt(dh) ; attn = softmax(dots)
    attn = where(attn > 0.01, attn, 0) ; out = attn @ v
    return out @ Wout + bout

Sharding over 8 NeuronCores: core c handles batch b = c//2 and head group
g = c%2 (4 of the 8 heads), producing a partial output projection for its
batch; host sums the two partials per batch and adds bout.

Numerics: fp16 two-limb (hi+lo) matmuls for the qk projection and the
attention logits (error ~1e-5, threshold-flip free vs fp32); exp in fp32 on
the Scalar engine (softmax without max subtraction: logits have unit
variance so exp is fp32-safe); the softmax denominator Z is accumulated on
the Tensor engine from two 16-bit limbs of E (fp16 RNE copy + bf16
residual, |dZ|/Z ~ 3e-7); the attn>0.01 compare is then fp32-exact against
c = 0.01*Z via a one-pass custom DVE select op; masked weights and V go
through the PE in bf16. Measured vs the CPU fp32 reference: absmax error
5.2e-4 (0.4% of the output absmax), zero threshold flips.
"""
import os
import sys
import functools

import numpy as np

for _p in ("/opt/trn_rl_repo", "/root/.axon_site", "/root/.axon_site/_ro/trn_rl_repo"):
    if os.path.isdir(_p) and _p not in sys.path:
        sys.path.append(_p)

import ml_dtypes
from contextlib import ExitStack

import concourse.bass as bass
import concourse.bacc as bacc
import concourse.mybir as mybir
import concourse.tile as tile
from concourse import bass_utils

FP32 = mybir.dt.float32
FP16 = mybir.dt.float16
BF16 = mybir.dt.bfloat16
ALU = mybir.AluOpType
AFT = mybir.ActivationFunctionType

BF16_TRUNC_CORR = 1.0 + 2.0 ** -8  # legacy (unused)


def _register_mask_op():
    """One-pass masked keep: out = in0 if in1 < in0 else 0.

    Registered through the documented custom-DVE extension point
    (dve_ops.OPS); used with in0 = E (fp32) and in1 = broadcast threshold.
    """
    from concourse.dve_spec import Spec, Src0, Src1, Zero, select
    from concourse import dve_ops as dops

    name = "MASK_KEEP_GT_ANT"
    for op in dops.OPS:
        if op.name == name:
            return op
    op = dops.DveOp(
        name,
        Spec(
            body=select(Src1 < Src0, Src0, Zero),
            reference=lambda in0, in1, s0, s1, imm2: np.where(
                in1 < in0, in0, 0.0).astype(np.float32),
        ),
        subdim=False,
        uops_sha={"v3": "d86f8416d0d7b042", "v4": "f70e64aee8639ca3"},
    )
    dops.OPS.append(op)
    dops._SUB_OPCODE_FOR_NAME[name] = dops._CUSTOM_DVE_ROW_BASE + len(dops.OPS) - 1
    dops.CUSTOM_DVE_SPECS[name] = op.spec
    return op


MASK_OP = _register_mask_op()


def emit_core_kernel(ctx, tc, io, n=2048, dim=512, hc=4, dh=64, qch=512):
    """Emit one core's program. io: dict name -> bass.AP (DRAM).

    hc: heads on this core. qch: query-chunk (free-dim of S^T tiles).
    """
    nc = tc.nc
    inner = hc * dh                 # 256
    NT = n // 128                   # x row tiles / key chunks
    KC = n // 128
    SG = 2 if qch * 4 >= 2048 else 1  # key chunks per S/E tile (PSUM banks)
    KC2 = KC // SG                  # E-tiles per (h,qc)
    QC = n // qch
    DC = dim // 128                 # contraction chunks of dim
    MH = inner // 128               # m-tiles of qT (and of kT)
    MQK = 2 * MH                    # m-tiles of stacked [q;k]T
    scale = dh ** -0.5

    # ---------------- constants ----------------
    cpool = ctx.enter_context(tc.tile_pool(name="consts", bufs=1))
    ident = cpool.tile([128, 128], FP16, tag="ident", name="ident")
    nc.sync.dma_start(ident[:], io["ident"][:])
    wqk_h = []
    wqk_x = []
    wv_h = []
    for c in range(DC):
        t = cpool.tile([128, 2 * inner], FP16, tag=f"wqkh{c}", name=f"wqkh{c}")
        nc.sync.dma_start(t[:], io["wqk_h"][c * 128:(c + 1) * 128, :])
        wqk_h.append(t)
        t = cpool.tile([128, inner], FP16, tag=f"wvh{c}", name=f"wvh{c}")
        nc.sync.dma_start(t[:], io["wv_h"][c * 128:(c + 1) * 128, :])
        wv_h.append(t)
    for c in range(2 * DC):
        t = cpool.tile([128, 2 * inner], FP16, tag=f"wqkx{c}", name=f"wqkx{c}")
        nc.sync.dma_start(t[:], io["wqk_x"][c * 128:(c + 1) * 128, :])
        wqk_x.append(t)
    wout = []
    for m in range(MH):
        t = cpool.tile([128, dim], BF16, tag=f"wout{m}", name=f"wout{m}")
        nc.sync.dma_start(t[:], io["wout_b"][m * 128:(m + 1) * 128, :])
        wout.append(t)
    bqk = []
    for m in range(MQK):
        t = cpool.tile([128, 1], FP32, tag=f"bqk{m}", name=f"bqk{m}")
        nc.sync.dma_start(t[:], io["bqk"][m * 128:(m + 1) * 128, :])
        bqk.append(t)
    bv_row = cpool.tile([1, inner], FP32, tag="bv", name="bv_row")
    nc.sync.dma_start(bv_row[:], io["bv"][:])
    ones_col_bf = cpool.tile([128, 1], BF16, tag="ones_col", name="ones_col")
    nc.vector.memset(ones_col_bf[:], 1.0)
    ones_col_f16 = cpool.tile([128, 1], FP16, tag="ones_col16", name="ones_col16")
    nc.vector.memset(ones_col_f16[:], 1.0)
    ones_row_f = cpool.tile([1, 128], FP32, tag="ones_row", name="ones_row")
    nc.vector.memset(ones_row_f[:], 1.0)

    # persistent activations
    apool = ctx.enter_context(tc.tile_pool(name="acts", bufs=1))
    qkT_h = [apool.tile([128, n], FP16, tag=f"qkTh{m}", name=f"qkTh{m}") for m in range(MQK)]
    qkT_l = [apool.tile([128, n], FP16, tag=f"qkTl{m}", name=f"qkTl{m}") for m in range(MQK)]
    V_sb = [apool.tile([128, inner], BF16, tag=f"V{t}", name=f"V{t}") for t in range(NT)]
    attnT = [apool.tile([128, n], FP32, tag=f"attnT{m}", name=f"attnT{m}") for m in range(MH)]
    attnB = [apool.tile([128, n], BF16, tag=f"attnB{m}", name=f"attnB{m}") for m in range(MH)]

    # ---------------- phase A: x -> xT (hi/lo fp16) ----------------
    with tc.tile_pool(name="xT", bufs=1) as xtp:
        xTh = [xtp.tile([128, n], FP16, tag=f"xTh{c}", name=f"xTh{c}") for c in range(DC)]
        xTl = [xtp.tile([128, n], FP16, tag=f"xTl{c}", name=f"xTl{c}") for c in range(DC)]
        with tc.tile_pool(name="xin", bufs=4) as xip, \
             tc.tile_pool(name="psA", bufs=4, space="PSUM") as psA:
            for nt in range(NT):
                for src, dsts in (("xh", xTh), ("xl", xTl)):
                    xt = xip.tile([128, dim], FP16, tag="xin")
                    nc.sync.dma_start(xt[:], io[src][nt * 128:(nt + 1) * 128, :])
                    for c in range(DC):
                        ps = psA.tile([128, 128], FP16, tag="psA")
                        nc.tensor.transpose(ps[:], xt[:, c * 128:(c + 1) * 128],
                                            ident[:])
                        nc.vector.tensor_copy(
                            dsts[c][:, nt * 128:(nt + 1) * 128], ps[:])

        # ---------------- phase B: projections ----------------
        nqs = min(512, n)
        with tc.tile_pool(name="psB", bufs=4, space="PSUM") as psB:
            # qkT = (Wqk^T x^T) as hi+lo fp16, with bias
            for m in range(MQK):
                for nq in range(n // nqs):
                    ps = psB.tile([128, nqs], FP32, tag="psB")
                    sl = slice(nq * nqs, (nq + 1) * nqs)
                    msl = slice(m * 128, (m + 1) * 128)
                    for c in range(DC):
                        nc.tensor.matmul(ps[:], wqk_h[c][:, msl],
                                         xTh[c][:, sl],
                                         start=(c == 0), stop=False)
                    for c2 in range(2 * DC):
                        rhs = xTh[c2][:, sl] if c2 < DC else xTl[c2 - DC][:, sl]
                        nc.tensor.matmul(ps[:], wqk_x[c2][:, msl], rhs,
                                         start=False, stop=(c2 == 2 * DC - 1))
                    nc.vector.tensor_scalar(qkT_h[m][:, sl], ps[:], bqk[m][:],
                                            None, ALU.add)
                    nc.vector.scalar_tensor_tensor(
                        qkT_l[m][:, sl], ps[:], bqk[m][:], qkT_h[m][:, sl],
                        ALU.add, ALU.subtract)
            # V natural [n, inner] in bf16, bias via rank-1 ones
            for nt in range(NT):
                ps = psB.tile([128, inner], FP32, tag="psBv")
                tsl = slice(nt * 128, (nt + 1) * 128)
                for c in range(DC):
                    nc.tensor.matmul(ps[:], xTh[c][:, tsl], wv_h[c][:],
                                     start=(c == 0), stop=False)
                nc.tensor.matmul(ps[:], ones_row_f[:], bv_row[:],
                                 start=False, stop=True)
                nc.vector.tensor_copy(V_sb[nt][:], ps[:])

    # ---------------- phase C: attention ----------------
    # Z is accumulated on the PE from two 16-bit limbs of E: an fp16 RNE
    # copy (11 bits, DVE cast at 2x) plus a bf16 residual (8 more bits,
    # computed on GPSIMD) -> |dZ|/Z ~ 3e-7, so the attn>0.01 compare sees
    # an effectively fp32-exact threshold.
    #
    # Software pipeline: iteration i+1's S/exp/limb/Z work is emitted
    # before iteration i's mask/PV tail so the PE's in-order queue never
    # stalls on DVE-produced mask tiles; Z matmuls are skewed two S-tiles
    # behind the exp that feeds them for the same reason.
    with tc.tile_pool(name="psS", bufs=2, space="PSUM") as psS, \
         tc.tile_pool(name="psZZ", bufs=2, space="PSUM") as psZZp, \
         tc.tile_pool(name="psCB", bufs=1, space="PSUM") as psCBp, \
         tc.tile_pool(name="psO", bufs=1, space="PSUM") as psOp, \
         tc.tile_pool(name="Epool", bufs=2 * KC2, space="SBUF") as Ep, \
         tc.tile_pool(name="limb", bufs=6) as lp, \
         tc.tile_pool(name="mp", bufs=2 * KC2, space="SBUF") as mp, \
         tc.tile_pool(name="crow", bufs=1) as crp:

        def flush_z2(jobs):
            """Residual-limb Z matmuls; deferred a full pipeline stage so the
            PE never waits on the GPSIMD-produced Er tiles."""
            for kt_, Er_, psZZ_ in jobs:
                for j in range(SG):
                    jsl = slice(j * qch, (j + 1) * qch)
                    nc.tensor.matmul(psZZ_[32:33, :], ones_col_bf[:],
                                     Er_[:, jsl],
                                     start=(kt_ == 0 and j == 0),
                                     stop=(kt_ == KC2 - 1 and j == SG - 1))
            jobs.clear()

        def stage_a(h, qc, z2_prev, bhead=None):
            """S^T matmuls, exp, Z limbs, Z accumulation for one (h, qc)."""
            mq, rq = h // 2, 64 * (h % 2)
            mk = MH + h // 2
            qsl_h = slice(rq, rq + 64)
            qsl = slice(qc * qch, (qc + 1) * qch)
            q_hi = qkT_h[mq][qsl_h, qsl]
            q_lo = qkT_l[mq][qsl_h, qsl]
            psZZ = psZZp.tile([33, qch], FP32, tag="ZZ")
            E_tiles = []
            pending_z = []
            z2_jobs = []

            def flush_z1(limit):
                while len(pending_z) > limit:
                    kt_, Eh_ = pending_z.pop(0)
                    for j in range(SG):
                        jsl = slice(j * qch, (j + 1) * qch)
                        nc.tensor.matmul(psZZ[0:1, :], ones_col_f16[:],
                                         Eh_[:, jsl],
                                         start=(kt_ == 0 and j == 0),
                                         stop=(kt_ == KC2 - 1 and j == SG - 1))

            for kt in range(KC2):
                ps = psS.tile([128, SG * qch], FP32, tag="S")
                for j in range(SG):
                    kc = SG * kt + j
                    ksl = slice(kc * 128, (kc + 1) * 128)
                    out = ps[:, j * qch:(j + 1) * qch]
                    k_hi = qkT_h[mk][qsl_h, ksl]
                    k_lo = qkT_l[mk][qsl_h, ksl]
                    nc.tensor.matmul(out, k_hi, q_hi, start=True, stop=False)
                    nc.tensor.matmul(out, k_lo, q_hi, start=False, stop=False)
                    nc.tensor.matmul(out, k_hi, q_lo, start=False, stop=True)
                if kt == 1 and z2_prev:
                    flush_z2(z2_prev)
                    if bhead is not None:
                        bhead()
                Et = Ep.tile([128, SG * qch], FP32, tag="E")
                nc.scalar.activation(Et[:], ps[:], AFT.Exp, scale=scale)
                E_tiles.append(Et)
                # two 16-bit limbs of E for the exact-Z matmuls
                Eh = lp.tile([128, SG * qch], FP16, tag="Eh")
                nc.vector.tensor_copy(Eh[:], Et[:])
                Er = lp.tile([128, SG * qch], BF16, tag="Er", bufs=10)
                nc.gpsimd.tensor_tensor(Er[:], Et[:], Eh[:], ALU.subtract)
                pending_z.append((kt, Eh))
                z2_jobs.append((kt, Er, psZZ))
                flush_z1(2)
            flush_z1(0)
            return E_tiles, psZZ, z2_jobs

        def stage_b_head(state):
            """Z finalize + threshold broadcast; emitted early (inside the
            next iteration's stage_a) so the PE/DVE see it promptly."""
            E_tiles, psZZ, _ = state
            z2_row = crp.tile([1, qch], FP32, tag="z2row")
            nc.scalar.activation(z2_row[:], psZZ[32:33, :], AFT.Copy)
            z_row = crp.tile([1, qch], FP32, tag="zrow")
            nc.vector.scalar_tensor_tensor(z_row[:], psZZ[0:1, :], 0.0,
                                           z2_row[:], ALU.add, ALU.add)
            c_row = crp.tile([1, qch], FP32, tag="crow")
            nc.vector.tensor_scalar(c_row[:], z_row[:], 0.01, None, ALU.mult)
            r_row = crp.tile([1, qch], FP32, tag="rrow")
            nc.vector.reciprocal_approx_fast(out=r_row[:], in_=z_row[:])
            psCB = psCBp.tile([128, qch], FP32, tag="CB")
            nc.tensor.matmul(psCB[:], ones_row_f[:], c_row[:],
                             start=True, stop=True)
            return psCB, r_row

        def stage_b(h, qc, state, head):
            """masks, PV, 1/Z scale for one (h, qc)."""
            E_tiles, psZZ, _ = state
            psCB, r_row = head
            mq, rq = h // 2, 64 * (h % 2)
            qsl_h = slice(rq, rq + 64)
            qsl = slice(qc * qch, (qc + 1) * qch)
            P_tiles = []
            for kt in range(KC2):
                Et = E_tiles[kt]
                for j in range(SG):
                    esl = Et[:, j * qch:(j + 1) * qch]
                    Pt = mp.tile([128, qch], BF16, tag="P")
                    nc.vector._custom_dve(MASK_OP, out=Pt[:], in0=esl,
                                          in1=psCB[:])
                    P_tiles.append(Pt)
            psO = psOp.tile([64, qch], FP32, tag="O")
            for kc in range(KC):
                nc.tensor.matmul(psO[:], V_sb[kc][:, h * dh:(h + 1) * dh],
                                 P_tiles[kc][:],
                                 start=(kc == 0), stop=(kc == KC - 1))
            nc.scalar.activation(attnT[mq][qsl_h, qsl], psO[:], AFT.Copy)
            # scale by 1/Z: broadcast r over the 64 head dims, multiply
            psR = psZZp.tile([64, qch], FP32, tag="ZZ")
            nc.tensor.matmul(psR[:], ones_row_f[:, :64], r_row[:],
                             start=True, stop=True)
            nc.vector.tensor_tensor(attnB[mq][qsl_h, qsl],
                                    attnT[mq][qsl_h, qsl], psR[:],
                                    ALU.mult)

        order = [(h, qc) for h in range(hc) for qc in range(QC)]
        prev = None
        z2_prev = []
        head_box = {}
        for hq in order:
            pstate = prev[1] if prev is not None else None
            bhead = (lambda s=pstate: head_box.__setitem__("h", stage_b_head(s))) \
                if pstate is not None else None
            state = stage_a(hq[0], hq[1], z2_prev, bhead)
            z2_prev = state[2]
            if prev is not None:
                stage_b(prev[0][0], prev[0][1], prev[1], head_box.pop("h"))
            prev = (hq, state)
        flush_z2(z2_prev)
        head = stage_b_head(prev[1])
        stage_b(prev[0][0], prev[0][1], prev[1], head)

    # ---------------- phase E: output projection ----------------
    with tc.tile_pool(name="psE", bufs=4, space="PSUM") as psE, \
         tc.tile_pool(name="ostage", bufs=4) as osp:
        for nt in range(NT):
            ps = psE.tile([128, dim], FP32, tag="psE")
            tsl = slice(nt * 128, (nt + 1) * 128)
            for m in range(MH):
                nc.tensor.matmul(ps[:], attnB[m][:, tsl], wout[m][:],
                                 start=(m == 0), stop=(m == MH - 1))
            ot = osp.tile([128, dim], FP32, tag="ostage")
            eng = nc.vector if nt % 2 == 0 else nc.scalar
            if eng is nc.scalar:
                nc.scalar.activation(ot[:], ps[:], AFT.Copy)
            else:
                nc.vector.tensor_copy(ot[:], ps[:])
            nc.sync.dma_start(io["out"][tsl, :], ot[:])


def build_program(n=2048, dim=512, hc=4, dh=64, qch=512):
    nc = bacc.Bacc(trn_type="TRN2", target_bir_lowering=False, debug=False)
    inner = hc * dh
    io = {}

    def din(name, shape, dt):
        io[name] = nc.dram_tensor(name, shape, dt, kind="ExternalInput").ap()

    din("xh", [n, dim], FP16)
    din("xl", [n, dim], FP16)
    din("wqk_h", [dim, 2 * inner], FP16)
    din("wqk_x", [2 * dim, 2 * inner], FP16)
    din("wv_h", [dim, inner], FP16)
    din("bqk", [2 * inner, 1], FP32)
    din("bv", [1, inner], FP32)
    din("wout_b", [inner, dim], BF16)
    din("ident", [128, 128], FP16)
    io["out"] = nc.dram_tensor("out", [n, dim], FP32, kind="ExternalOutput").ap()

    with tile.TileContext(nc) as tc:
        with ExitStack() as ctx:
            emit_core_kernel(ctx, tc, io, n=n, dim=dim, hc=hc, dh=dh, qch=qch)
    nc.compile()
    return nc


def make_core_inputs(x_b, Wq, Wk, Wv, bq, bk, bv, Wout_g, n=2048, dim=512,
                     hc=4, dh=64):
    """Host-side prep of one core's input dict (numpy, correct dtypes)."""
    f16 = np.float16
    inner = hc * dh
    xh = x_b.astype(f16)
    xl = (x_b - xh.astype(np.float32)).astype(f16)
    wqk = np.concatenate([Wq, Wk], axis=1)              # [dim, 2*inner]
    wqk_hi = wqk.astype(f16)
    wqk_lo = (wqk - wqk_hi.astype(np.float32)).astype(f16)
    wqk_x = np.concatenate([wqk_lo, wqk_hi], axis=0)    # [2*dim, 2*inner]
    return {
        "xh": xh, "xl": xl,
        "wqk_h": wqk_hi, "wqk_x": wqk_x,
        "wv_h": Wv.astype(f16),
        "bqk": np.concatenate([bq, bk]).reshape(2 * inner, 1).astype(np.float32),
        "bv": bv.reshape(1, inner).astype(np.float32),
        "wout_b": Wout_g.astype(ml_dtypes.bfloat16),
        "ident": np.eye(128, dtype=f16),
    }


@functools.lru_cache(maxsize=1)
def _cached_program():
    return build_program()


def kernel(x, Wqkv, bqkv, Wout, bout):
    x = np.asarray(x, dtype=np.float32)
    Wqkv = np.asarray(Wqkv, dtype=np.float32)
    bqkv = np.asarray(bqkv, dtype=np.float32)
    Wout = np.asarray(Wout, dtype=np.float32)
    bout = np.asarray(bout, dtype=np.float32)

    b, n, dim = x.shape
    H, dh = 8, 64
    inner = H * dh
    hc = 4  # heads per core
    Wq, Wk, Wv = Wqkv[:, :inner], Wqkv[:, inner:2 * inner], Wqkv[:, 2 * inner:]
    bq, bk, bv = bqkv[:inner], bqkv[inner:2 * inner], bqkv[2 * inner:]

    in_maps = []
    for c in range(8):
        bb, g = c // 2, c % 2
        hsl = slice(g * hc * dh, (g + 1) * hc * dh)
        in_maps.append(make_core_inputs(
            x[bb], Wq[:, hsl], Wk[:, hsl], Wv[:, hsl],
            bq[hsl], bk[hsl], bv[hsl], Wout[hsl, :],
            n=n, dim=dim, hc=hc, dh=dh))

    nc = _cached_program()
    res = bass_utils.run_bass_kernel_spmd(nc, in_maps, core_ids=list(range(8)))
    global LAST_RESULTS
    LAST_RESULTS = res
    out = np.empty((b, n, dim), dtype=np.float32)
    for bb in range(b):
        out[bb] = res.results[2 * bb]["out"] + res.results[2 * bb + 1]["out"] \
            + bout
    return out



# revision 4
# speedup vs baseline: 1.6322x; 1.6322x over previous
"""Trainium2 Bass kernel for thresholded multi-head attention.

Computes, for x:[b,n,dim] with b=4, n=2048, dim=512, heads=8, dh=64:
    qkv = x @ Wqkv + bqkv ; split q,k,v per head
    dots = q k^T / sqrt(dh) ; attn = softmax(dots)
    attn = where(attn > 0.01, attn, 0) ; out = attn @ v
    return out @ Wout + bout

Sharding over 8 NeuronCores: core c handles batch b = c//2 and head group
g = c%2 (4 of the 8 heads), producing a partial output projection for its
batch; host sums the two partials per batch and adds bout.

v2 rework (PE instruction-efficiency focused):
  - x is transposed to xT on the host (pure layout), removing the on-device
    PE transpose phase.
  - S^T matmuls use limb-stacked contraction: K2 = [k_hi; k_lo] (c=128)
    stationary, streaming [q_hi;q_hi] then [q_lo;q_lo], i.e. 2 matmuls per
    S tile instead of 3 c=64 matmuls (and slightly MORE accurate: the
    lo*lo term is included).
  - Z (softmax denominator) comes from fp32 ones-matmuls over the fp32 E
    tiles, quad-packed into the PE's four 32-column sub-array groups via
    tile_position, so the four Z streams run concurrently. This removes
    the fp16/bf16 Z-limb machinery (DVE cast + GPSIMD residual) entirely.
  - The threshold row c = 0.01*Z is built by one matmul whose stationary
    is a uniform 0.01/32 [128,128] tile: it simultaneously reduces the 4
    Z bands and broadcasts 0.01*Z to all 128 partitions.
  - attn>0.01 masking stays the one-pass custom DVE select op, fp32-exact
    against c.
"""
import os
import sys
import functools

import numpy as np

for _p in ("/opt/trn_rl_repo", "/root/.axon_site", "/root/.axon_site/_ro/trn_rl_repo"):
    if os.path.isdir(_p) and _p not in sys.path:
        sys.path.append(_p)

import ml_dtypes
from contextlib import ExitStack

import concourse.bass as bass
import concourse.bacc as bacc
import concourse.mybir as mybir
import concourse.tile as tile
from concourse import bass_utils

FP32 = mybir.dt.float32
FP16 = mybir.dt.float16
BF16 = mybir.dt.bfloat16
ALU = mybir.AluOpType
AFT = mybir.ActivationFunctionType


def _register_mask_op():
    """One-pass masked keep: out = in0 if in1 < in0 else 0.

    Registered through the documented custom-DVE extension point
    (dve_ops.OPS); used with in0 = E (fp32) and in1 = broadcast threshold.
    """
    from concourse.dve_spec import Spec, Src0, Src1, Zero, select
    from concourse import dve_ops as dops

    name = "MASK_KEEP_GT_ANT"
    for op in dops.OPS:
        if op.name == name:
            return op
    op = dops.DveOp(
        name,
        Spec(
            body=select(Src1 < Src0, Src0, Zero),
            reference=lambda in0, in1, s0, s1, imm2: np.where(
                in1 < in0, in0, 0.0).astype(np.float32),
        ),
        subdim=False,
        uops_sha={"v3": "d86f8416d0d7b042", "v4": "f70e64aee8639ca3"},
    )
    dops.OPS.append(op)
    dops._SUB_OPCODE_FOR_NAME[name] = dops._CUSTOM_DVE_ROW_BASE + len(dops.OPS) - 1
    dops.CUSTOM_DVE_SPECS[name] = op.spec
    return op


MASK_OP = _register_mask_op()


def emit_core_kernel(ctx, tc, io, n=2048, dim=512, hc=4, dh=64, qch=512):
    """Emit one core's program. io: dict name -> bass.AP (DRAM)."""
    nc = tc.nc
    inner = hc * dh                 # 256
    NT = n // 128                   # 16 row tiles
    KC = n // 128                   # 16 key chunks
    KT = KC // 2                    # 8 S-psum tiles per stage (2 chunks each)
    QC = n // qch                   # 4 query chunks
    DC = dim // 128                 # 4 contraction chunks of dim
    MH = inner // 128               # 2 m-tiles of inner dims
    MQK = 2 * MH                    # 4 projection m-tiles ([q;k])
    scale = dh ** -0.5

    # ---------------- constants ----------------
    cpool = ctx.enter_context(tc.tile_pool(name="consts", bufs=1))
    wqk_h = []
    wqk_x = []
    wv_h = []
    for c in range(DC):
        t = cpool.tile([128, 2 * inner], FP16, tag=f"wqkh{c}", name=f"wqkh{c}")
        nc.sync.dma_start(t[:], io["wqk_h"][c * 128:(c + 1) * 128, :])
        wqk_h.append(t)
        t = cpool.tile([128, inner], FP16, tag=f"wvh{c}", name=f"wvh{c}")
        nc.sync.dma_start(t[:], io["wv_h"][c * 128:(c + 1) * 128, :])
        wv_h.append(t)
    for c in range(2 * DC):
        t = cpool.tile([128, 2 * inner], FP16, tag=f"wqkx{c}", name=f"wqkx{c}")
        nc.sync.dma_start(t[:], io["wqk_x"][c * 128:(c + 1) * 128, :])
        wqk_x.append(t)
    wout = []
    for m in range(MH):
        t = cpool.tile([128, dim], BF16, tag=f"wout{m}", name=f"wout{m}")
        nc.sync.dma_start(t[:], io["wout_b"][m * 128:(m + 1) * 128, :])
        wout.append(t)
    bqk = []
    for m in range(MQK):
        t = cpool.tile([128, 1], FP32, tag=f"bqk{m}", name=f"bqk{m}")
        nc.sync.dma_start(t[:], io["bqk"][m * 128:(m + 1) * 128, :])
        bqk.append(t)
    bv_row = cpool.tile([1, inner], FP32, tag="bv", name="bv_row")
    nc.sync.dma_start(bv_row[:], io["bv"][:])
    ones_row_f = cpool.tile([1, 128], FP32, tag="ones_row", name="ones_row")
    nc.vector.memset(ones_row_f[:], 1.0)
    # Z quad stationary: every output partition of a 32-col group gets the
    # full column sum of the streamed E tile.
    ones32_f = cpool.tile([128, 32], FP32, tag="ones32", name="ones32")
    nc.vector.memset(ones32_f[:], 1.0)
    # threshold stationary: c = 0.01 * (Z0+Z1+Z2+Z3) broadcast to 128 rows,
    # where each Z band is replicated over 32 partitions of zq.
    cq128 = cpool.tile([128, 128], FP32, tag="cq128", name="cq128")
    nc.vector.memset(cq128[:], 0.01 / 32.0)
    # 1/Z broadcast stationary for the output scale (r = 0.01 * rc).
    c01_row = cpool.tile([1, 64], FP32, tag="c01row", name="c01row")
    nc.vector.memset(c01_row[:], 0.01)

    # persistent activations
    apool = ctx.enter_context(tc.tile_pool(name="acts", bufs=1))
    # per head: K2 = [k_hi; k_lo] limbs stacked on partitions; Qh/Ql = the
    # q hi/lo limbs duplicated over both partition halves.
    K2 = [apool.tile([128, n], FP16, tag=f"K2_{h}", name=f"K2_{h}") for h in range(hc)]
    Qh = [apool.tile([128, n], FP16, tag=f"Qh_{h}", name=f"Qh_{h}") for h in range(hc)]
    Ql = [apool.tile([128, n], FP16, tag=f"Ql_{h}", name=f"Ql_{h}") for h in range(hc)]
    V_sb = [apool.tile([128, inner], BF16, tag=f"V{t}", name=f"V{t}") for t in range(NT)]
    attnT = [apool.tile([128, n], FP32, tag=f"attnT{m}", name=f"attnT{m}") for m in range(MH)]
    attnB = [apool.tile([128, n], BF16, tag=f"attnB{m}", name=f"attnB{m}") for m in range(MH)]

    # ---------------- phase B: projections ----------------
    nqs = 512
    NQ = n // nqs
    with tc.tile_pool(name="xT", bufs=1) as xtp:
        xTh = [xtp.tile([128, n], FP16, tag=f"xTh{c}", name=f"xTh{c}") for c in range(DC)]
        xTl = [xtp.tile([128, n], FP16, tag=f"xTl{c}", name=f"xTl{c}") for c in range(DC)]
        # DMA xT in query-column slices so the first projection tiles can
        # start before the whole 4MB lands.
        for nq in range(NQ):
            sl = slice(nq * nqs, (nq + 1) * nqs)
            for c in range(DC):
                rsl = slice(c * 128, (c + 1) * 128)
                nc.sync.dma_start(xTh[c][:, sl], io["xh"][rsl, sl])
                nc.sync.dma_start(xTl[c][:, sl], io["xl"][rsl, sl])

        with tc.tile_pool(name="psB", bufs=4, space="PSUM") as psB, \
             tc.tile_pool(name="psBv", bufs=2, space="PSUM") as psBv, \
             tc.tile_pool(name="kscr", bufs=4) as kscr:
            for nq in range(NQ):
                sl = slice(nq * nqs, (nq + 1) * nqs)
                for m in range(MQK):
                    ps = psB.tile([128, nqs], FP32, tag="psB")
                    msl = slice(m * 128, (m + 1) * 128)
                    for c in range(DC):
                        nc.tensor.matmul(ps[:], wqk_h[c][:, msl],
                                         xTh[c][:, sl],
                                         start=(c == 0), stop=False)
                    for c2 in range(2 * DC):
                        rhs = xTh[c2][:, sl] if c2 < DC else xTl[c2 - DC][:, sl]
                        nc.tensor.matmul(ps[:], wqk_x[c2][:, msl], rhs,
                                         start=False, stop=(c2 == 2 * DC - 1))
                    # unpack into per-head limb layouts. DVE ops stay
                    # partition-aligned (walrus requirement); the cross-half
                    # duplicates go through ACT copies, which may shift
                    # partitions.
                    for s in range(2):
                        rsl = slice(64 * s, 64 * s + 64)
                        osl = slice(64 * (1 - s), 64 * (1 - s) + 64)
                        bsub = bqk[m][rsl]
                        if m < MH:          # q part: heads 2m, 2m+1
                            hh = 2 * m + s
                            nc.vector.tensor_scalar(
                                Qh[hh][rsl, sl], ps[rsl, :], bsub, None, ALU.add)
                            nc.vector.scalar_tensor_tensor(
                                Ql[hh][rsl, sl], ps[rsl, :], bsub,
                                Qh[hh][rsl, sl], ALU.add, ALU.subtract)
                            nc.scalar.activation(Qh[hh][osl, sl],
                                                 Qh[hh][rsl, sl], AFT.Copy)
                            nc.scalar.activation(Ql[hh][osl, sl],
                                                 Ql[hh][rsl, sl], AFT.Copy)
                        else:               # k part: heads 2(m-MH), +1
                            hh = 2 * (m - MH) + s
                            if s == 0:
                                # hi lands aligned at rows 0:64; lo via scratch
                                klo = kscr.tile([128, nqs], FP16, tag="kscr")
                                nc.vector.tensor_scalar(
                                    K2[hh][0:64, sl], ps[0:64, :], bsub,
                                    None, ALU.add)
                                nc.vector.scalar_tensor_tensor(
                                    klo[0:64, :], ps[0:64, :], bsub,
                                    K2[hh][0:64, sl], ALU.add, ALU.subtract)
                                nc.scalar.activation(K2[hh][64:128, sl],
                                                     klo[0:64, :], AFT.Copy)
                            else:
                                # lo lands aligned at rows 64:128; hi via scratch
                                khi = kscr.tile([128, nqs], FP16, tag="kscr")
                                nc.vector.tensor_scalar(
                                    khi[64:128, :], ps[64:128, :], bsub,
                                    None, ALU.add)
                                nc.vector.scalar_tensor_tensor(
                                    K2[hh][64:128, sl], ps[64:128, :], bsub,
                                    khi[64:128, :], ALU.add, ALU.subtract)
                                nc.scalar.activation(K2[hh][0:64, sl],
                                                     khi[64:128, :], AFT.Copy)
                # V natural [n, inner] in bf16, bias via rank-1 ones
                for nt in range(4 * nq, 4 * nq + 4):
                    ps = psBv.tile([128, inner], FP32, tag="psBv")
                    tsl = slice(nt * 128, (nt + 1) * 128)
                    for c in range(DC):
                        nc.tensor.matmul(ps[:], xTh[c][:, tsl], wv_h[c][:],
                                         start=(c == 0), stop=False)
                    nc.tensor.matmul(ps[:], ones_row_f[:], bv_row[:],
                                     start=False, stop=True)
                    nc.vector.tensor_copy(V_sb[nt][:], ps[:])

    # ---------------- phase C: attention ----------------
    with tc.tile_pool(name="psS", bufs=2, space="PSUM") as psS, \
         tc.tile_pool(name="psZ", bufs=2, space="PSUM") as psZp, \
         tc.tile_pool(name="psCB", bufs=1, space="PSUM") as psCBp, \
         tc.tile_pool(name="psOR", bufs=1, space="PSUM") as psORp, \
         tc.tile_pool(name="Epool", bufs=2 * KT, space="SBUF") as Ep, \
         tc.tile_pool(name="mp", bufs=KC + 2, space="SBUF") as mp, \
         tc.tile_pool(name="zqp", bufs=2) as zqp, \
         tc.tile_pool(name="rcp", bufs=2) as rcp:

        def s_block(h, qc, psZ_t, E_tiles, kt):
            """S matmuls + exp for one psS tile (two key chunks)."""
            qsl = slice(qc * qch, (qc + 1) * qch)
            ps = psS.tile([128, 2 * qch], FP32, tag="S")
            for j in range(2):
                kc = 2 * kt + j
                ksl = slice(kc * 128, (kc + 1) * 128)
                out = ps[:, j * qch:(j + 1) * qch]
                lhsT = K2[h][:, ksl]
                nc.tensor.matmul(out, lhsT, Qh[h][:, qsl],
                                 start=True, stop=False)
                nc.tensor.matmul(out, lhsT, Ql[h][:, qsl],
                                 start=False, stop=True)
            Et = Ep.tile([128, 2 * qch], FP32, tag="E")
            nc.scalar.activation(Et[:], ps[:], AFT.Exp, scale=scale)
            E_tiles.append(Et)

        def z_quad(psZ_t, E_tiles, q):
            """One concurrent quad of fp32 Z matmuls (key chunks 4q..4q+3)."""
            for i in range(4):
                kc = 4 * q + i
                kt, j = kc // 2, kc % 2
                g = kc % 4
                nc.tensor.matmul(
                    psZ_t[32 * g:32 * g + 32, :], ones32_f[:],
                    E_tiles[kt][:, j * qch:(j + 1) * qch],
                    start=(kc < 4), stop=(kc >= KC - 4),
                    tile_position=(0, 32 * g))

        def stage_a(h, qc, bhead=None):
            """S^T matmuls, exp, Z accumulation for one (h, qc)."""
            psZ_t = psZp.tile([128, qch], FP32, tag="Z")
            E_tiles = []
            s_block(h, qc, psZ_t, E_tiles, 0)
            s_block(h, qc, psZ_t, E_tiles, 1)
            if bhead is not None:
                bhead()
            s_block(h, qc, psZ_t, E_tiles, 2)
            z_quad(psZ_t, E_tiles, 0)
            s_block(h, qc, psZ_t, E_tiles, 3)
            s_block(h, qc, psZ_t, E_tiles, 4)
            z_quad(psZ_t, E_tiles, 1)
            s_block(h, qc, psZ_t, E_tiles, 5)
            s_block(h, qc, psZ_t, E_tiles, 6)
            z_quad(psZ_t, E_tiles, 2)
            s_block(h, qc, psZ_t, E_tiles, 7)
            z_quad(psZ_t, E_tiles, 3)
            return E_tiles, psZ_t

        def stage_b_head(state):
            """Z finalize: zq copy, threshold broadcast, reciprocal row."""
            E_tiles, psZ_t = state
            zq = zqp.tile([128, qch], FP32, tag="zq")
            nc.vector.tensor_copy(zq[:], psZ_t[:])
            psCB = psCBp.tile([128, qch], FP32, tag="CB")
            nc.tensor.matmul(psCB[:], cq128[:], zq[:], start=True, stop=True)
            rc = rcp.tile([1, qch], FP32, tag="rc")
            nc.vector.reciprocal_approx_fast(out=rc[:], in_=psCB[0:1, :])
            return psCB, rc

        def stage_b(h, qc, state, head):
            """masks, PV, 1/Z scale for one (h, qc)."""
            E_tiles, _ = state
            psCB, rc = head
            mq, rq = h // 2, 64 * (h % 2)
            qsl_h = slice(rq, rq + 64)
            qsl = slice(qc * qch, (qc + 1) * qch)
            P_tiles = []
            for kt in range(KT):
                Et = E_tiles[kt]
                for j in range(2):
                    esl = Et[:, j * qch:(j + 1) * qch]
                    Pt = mp.tile([128, qch], BF16, tag="P")
                    nc.vector._custom_dve(MASK_OP, out=Pt[:], in0=esl,
                                          in1=psCB[:])
                    P_tiles.append(Pt)
            psOR = psORp.tile([128, qch], FP32, tag="OR")
            for kc in range(KC):
                nc.tensor.matmul(psOR[0:64, :],
                                 V_sb[kc][:, h * dh:(h + 1) * dh],
                                 P_tiles[kc][:],
                                 start=(kc == 0), stop=(kc == KC - 1))
            # r = 0.01 * rc = 1/Z, broadcast over the 64 head dims
            nc.tensor.matmul(psOR[64:128, :], c01_row[:], rc[:],
                             start=True, stop=True)
            nc.scalar.activation(attnT[mq][qsl_h, qsl], psOR[0:64, :], AFT.Copy)
            nc.vector.tensor_tensor(attnB[mq][qsl_h, qsl],
                                    attnT[mq][qsl_h, qsl], psOR[64:128, :],
                                    ALU.mult)

        order = [(h, qc) for h in range(hc) for qc in range(QC)]
        prev = None
        head_box = {}
        for hq in order:
            pstate = prev[1] if prev is not None else None
            bhead = (lambda s=pstate: head_box.__setitem__("h", stage_b_head(s))) \
                if pstate is not None else None
            state = stage_a(hq[0], hq[1], bhead)
            if prev is not None:
                stage_b(prev[0][0], prev[0][1], prev[1], head_box.pop("h"))
            prev = (hq, state)
        head = stage_b_head(prev[1])
        stage_b(prev[0][0], prev[0][1], prev[1], head)

    # ---------------- phase E: output projection ----------------
    with tc.tile_pool(name="psE", bufs=4, space="PSUM") as psE, \
         tc.tile_pool(name="ostage", bufs=4) as osp:
        for nt in range(NT):
            ps = psE.tile([128, dim], FP32, tag="psE")
            tsl = slice(nt * 128, (nt + 1) * 128)
            for m in range(MH):
                nc.tensor.matmul(ps[:], attnB[m][:, tsl], wout[m][:],
                                 start=(m == 0), stop=(m == MH - 1))
            ot = osp.tile([128, dim], FP32, tag="ostage")
            if nt % 2 == 0:
                nc.vector.tensor_copy(ot[:], ps[:])
            else:
                nc.scalar.activation(ot[:], ps[:], AFT.Copy)
            nc.sync.dma_start(io["out"][tsl, :], ot[:])


def build_program(n=2048, dim=512, hc=4, dh=64, qch=512):
    nc = bacc.Bacc(trn_type="TRN2", target_bir_lowering=False, debug=False)
    inner = hc * dh
    io = {}

    def din(name, shape, dt):
        io[name] = nc.dram_tensor(name, shape, dt, kind="ExternalInput").ap()

    din("xh", [dim, n], FP16)
    din("xl", [dim, n], FP16)
    din("wqk_h", [dim, 2 * inner], FP16)
    din("wqk_x", [2 * dim, 2 * inner], FP16)
    din("wv_h", [dim, inner], FP16)
    din("bqk", [2 * inner, 1], FP32)
    din("bv", [1, inner], FP32)
    din("wout_b", [inner, dim], BF16)
    io["out"] = nc.dram_tensor("out", [n, dim], FP32, kind="ExternalOutput").ap()

    with tile.TileContext(nc) as tc:
        with ExitStack() as ctx:
            emit_core_kernel(ctx, tc, io, n=n, dim=dim, hc=hc, dh=dh, qch=qch)
    nc.compile()
    return nc


def make_core_inputs(x_b, Wq, Wk, Wv, bq, bk, bv, Wout_g, n=2048, dim=512,
                     hc=4, dh=64):
    """Host-side prep of one core's input dict (numpy, correct dtypes)."""
    f16 = np.float16
    inner = hc * dh
    xT = np.ascontiguousarray(x_b.T)                    # [dim, n]
    xh = xT.astype(f16)
    xl = (xT - xh.astype(np.float32)).astype(f16)
    wqk = np.concatenate([Wq, Wk], axis=1)              # [dim, 2*inner]
    wqk_hi = wqk.astype(f16)
    wqk_lo = (wqk - wqk_hi.astype(np.float32)).astype(f16)
    wqk_x = np.concatenate([wqk_lo, wqk_hi], axis=0)    # [2*dim, 2*inner]
    return {
        "xh": xh, "xl": xl,
        "wqk_h": wqk_hi, "wqk_x": wqk_x,
        "wv_h": Wv.astype(f16),
        "bqk": np.concatenate([bq, bk]).reshape(2 * inner, 1).astype(np.float32),
        "bv": bv.reshape(1, inner).astype(np.float32),
        "wout_b": Wout_g.astype(ml_dtypes.bfloat16),
    }


@functools.lru_cache(maxsize=1)
def _cached_program():
    return build_program()


def kernel(x, Wqkv, bqkv, Wout, bout):
    x = np.asarray(x, dtype=np.float32)
    Wqkv = np.asarray(Wqkv, dtype=np.float32)
    bqkv = np.asarray(bqkv, dtype=np.float32)
    Wout = np.asarray(Wout, dtype=np.float32)
    bout = np.asarray(bout, dtype=np.float32)

    b, n, dim = x.shape
    H, dh = 8, 64
    inner = H * dh
    hc = 4  # heads per core
    Wq, Wk, Wv = Wqkv[:, :inner], Wqkv[:, inner:2 * inner], Wqkv[:, 2 * inner:]
    bq, bk, bv = bqkv[:inner], bqkv[inner:2 * inner], bqkv[2 * inner:]

    in_maps = []
    for c in range(8):
        bb, g = c // 2, c % 2
        hsl = slice(g * hc * dh, (g + 1) * hc * dh)
        in_maps.append(make_core_inputs(
            x[bb], Wq[:, hsl], Wk[:, hsl], Wv[:, hsl],
            bq[hsl], bk[hsl], bv[hsl], Wout[hsl, :],
            n=n, dim=dim, hc=hc, dh=dh))

    nc = _cached_program()
    res = bass_utils.run_bass_kernel_spmd(nc, in_maps, core_ids=list(range(8)))
    global LAST_RESULTS
    LAST_RESULTS = res
    out = np.empty((b, n, dim), dtype=np.float32)
    for bb in range(b):
        out[bb] = res.results[2 * bb]["out"] + res.results[2 * bb + 1]["out"] \
            + bout
    return out


# revision 8
# speedup vs baseline: 1.8146x; 1.1117x over previous
"""Trainium2 Bass kernel for thresholded multi-head attention.

Computes, for x:[b,n,dim] with b=4, n=2048, dim=512, heads=8, dh=64:
    qkv = x @ Wqkv + bqkv ; split q,k,v per head
    dots = q k^T / sqrt(dh) ; attn = softmax(dots)
    attn = where(attn > 0.01, attn, 0) ; out = attn @ v
    return out @ Wout + bout

Sharding over 8 NeuronCores: core c handles batch b = c//2 and head group
g = c%2 (4 of the 8 heads), producing a partial output projection for its
batch; host sums the two partials per batch and adds bout.

v3 (PE instruction-efficiency + engine balance):
  - x transposed on the host; no on-device transpose phase.
  - S^T via limb-stacked contraction (K2 = [k_hi;k_lo] stationary, two
    f=512 streams [q_hi;q_hi] / [q_lo;q_lo]).
  - Z: GPSIMD folds E tiles pairwise (fp32, exact), then 8 fp32
    ones-matmuls per stage, quad-packed into the four 32-column PE
    sub-array groups (tile_position) so they run concurrently. Z matmuls
    are emitted one stage late so the folds are always ready.
  - threshold c = 0.01*Z via one matmul with uniform 0.01/32 stationary
    (reduces the 4 Z bands + broadcasts in one shot).
  - attn>0.01 mask: one-pass custom DVE select at f=1024 (threshold read
    through a stride-0 broadcast AP).
  - PV matmuls (M=64) of two consecutive stages are column-paired into
    one [128,512] PSUM tile via the PE's 64-column groups -> ~2x PV.
  - 1/Z scale: GPSIMD partition_broadcast of the reciprocal row + one
    in-place DVE scalar_tensor_tensor (x0.01 folded in as immediate).
"""
import os
import sys
import functools

import numpy as np

for _p in ("/opt/trn_rl_repo", "/root/.axon_site", "/root/.axon_site/_ro/trn_rl_repo"):
    if os.path.isdir(_p) and _p not in sys.path:
        sys.path.append(_p)

import ml_dtypes
from contextlib import ExitStack

import concourse.bass as bass
import concourse.bacc as bacc
import concourse.mybir as mybir
import concourse.tile as tile
from concourse import bass_utils

FP32 = mybir.dt.float32
FP16 = mybir.dt.float16
BF16 = mybir.dt.bfloat16
ALU = mybir.AluOpType
AFT = mybir.ActivationFunctionType


def _register_mask_op():
    """One-pass masked keep: out = in0 if in1 < in0 else 0."""
    from concourse.dve_spec import Spec, Src0, Src1, Zero, select
    from concourse import dve_ops as dops

    name = "MASK_KEEP_GT_ANT"
    for op in dops.OPS:
        if op.name == name:
            return op
    op = dops.DveOp(
        name,
        Spec(
            body=select(Src1 < Src0, Src0, Zero),
            reference=lambda in0, in1, s0, s1, imm2: np.where(
                in1 < in0, in0, 0.0).astype(np.float32),
        ),
        subdim=False,
        uops_sha={"v3": "d86f8416d0d7b042", "v4": "f70e64aee8639ca3"},
    )
    dops.OPS.append(op)
    dops._SUB_OPCODE_FOR_NAME[name] = dops._CUSTOM_DVE_ROW_BASE + len(dops.OPS) - 1
    dops.CUSTOM_DVE_SPECS[name] = op.spec
    return op


MASK_OP = _register_mask_op()


def emit_core_kernel(ctx, tc, io, n=2048, dim=512, hc=4, dh=64, qch=512):
    """Emit one core's program. io: dict name -> bass.AP (DRAM)."""
    nc = tc.nc
    inner = hc * dh                 # 256
    NT = n // 128                   # 16 row tiles
    KC = n // 128                   # 16 key chunks
    KT = KC // 2                    # 8 S-psum tiles per stage
    QC = n // qch                   # 4 query chunks
    DC = dim // 128                 # 4 contraction chunks of dim
    MH = inner // 128               # 2 m-tiles of inner dims
    MQK = 2 * MH                    # 4 projection m-tiles ([q;k])
    scale = dh ** -0.5

    # ---------------- persistent constants ----------------
    cpool = ctx.enter_context(tc.tile_pool(name="consts", bufs=1))
    wout = []
    for m in range(MH):
        t = cpool.tile([128, dim], BF16, tag=f"wout{m}", name=f"wout{m}")
        nc.sync.dma_start(t[:], io["wout_b"][m * 128:(m + 1) * 128, :])
        wout.append(t)
    bqk = []
    for m in range(MQK):
        t = cpool.tile([128, 1], FP32, tag=f"bqk{m}", name=f"bqk{m}")
        nc.sync.dma_start(t[:], io["bqk"][m * 128:(m + 1) * 128, :])
        bqk.append(t)
    bv_row = cpool.tile([1, inner], FP32, tag="bv", name="bv_row")
    nc.sync.dma_start(bv_row[:], io["bv"][:])
    ones_row_f = cpool.tile([1, 128], FP32, tag="ones_row", name="ones_row")
    nc.vector.memset(ones_row_f[:], 1.0)
    ones32_f = cpool.tile([128, 32], FP32, tag="ones32", name="ones32")
    nc.vector.memset(ones32_f[:], 1.0)
    cq128 = cpool.tile([128, 128], FP32, tag="cq128", name="cq128")
    nc.vector.memset(cq128[:], 0.01 / 32.0)

    # persistent activations
    apool = ctx.enter_context(tc.tile_pool(name="acts", bufs=1))
    K2 = [apool.tile([128, n], FP16, tag=f"K2_{h}", name=f"K2_{h}") for h in range(hc)]
    Qh = [apool.tile([128, n], FP16, tag=f"Qh_{h}", name=f"Qh_{h}") for h in range(hc)]
    Ql = [apool.tile([128, n], FP16, tag=f"Ql_{h}", name=f"Ql_{h}") for h in range(hc)]
    V_sb = [apool.tile([128, inner], BF16, tag=f"V{t}", name=f"V{t}") for t in range(NT)]
    attnB = [apool.tile([128, n], BF16, tag=f"attnB{m}", name=f"attnB{m}") for m in range(MH)]

    # ---------------- phase B: projections ----------------
    nqs = 512
    NQ = n // nqs
    with tc.tile_pool(name="xT", bufs=1) as xtp, \
         tc.tile_pool(name="wB", bufs=1) as wbp:
        wqk_h = []
        wqk_x = []
        wv_h = []
        for c in range(DC):
            t = wbp.tile([128, 2 * inner], FP16, tag=f"wqkh{c}", name=f"wqkh{c}")
            nc.sync.dma_start(t[:], io["wqk_h"][c * 128:(c + 1) * 128, :])
            wqk_h.append(t)
            t = wbp.tile([128, inner], FP16, tag=f"wvh{c}", name=f"wvh{c}")
            nc.sync.dma_start(t[:], io["wv_h"][c * 128:(c + 1) * 128, :])
            wv_h.append(t)
        for c in range(2 * DC):
            t = wbp.tile([128, 2 * inner], FP16, tag=f"wqkx{c}", name=f"wqkx{c}")
            nc.sync.dma_start(t[:], io["wqk_x"][c * 128:(c + 1) * 128, :])
            wqk_x.append(t)
        xTh = [xtp.tile([128, n], FP16, tag=f"xTh{c}", name=f"xTh{c}") for c in range(DC)]
        xTl = [xtp.tile([128, n], FP16, tag=f"xTl{c}", name=f"xTl{c}") for c in range(DC)]
        for nq in range(NQ):
            sl = slice(nq * nqs, (nq + 1) * nqs)
            for c in range(DC):
                rsl = slice(c * 128, (c + 1) * 128)
                nc.sync.dma_start(xTh[c][:, sl], io["xh"][rsl, sl])
                nc.sync.dma_start(xTl[c][:, sl], io["xl"][rsl, sl])

        with tc.tile_pool(name="psB", bufs=4, space="PSUM") as psB, \
             tc.tile_pool(name="psBv", bufs=2, space="PSUM") as psBv, \
             tc.tile_pool(name="kscr", bufs=4) as kscr:
            for nq in range(NQ):
                sl = slice(nq * nqs, (nq + 1) * nqs)
                for m in range(MQK):
                    ps = psB.tile([128, nqs], FP32, tag="psB", name=f"psB{nq}_{m}")
                    msl = slice(m * 128, (m + 1) * 128)
                    for c in range(DC):
                        nc.tensor.matmul(ps[:], wqk_h[c][:, msl],
                                         xTh[c][:, sl],
                                         start=(c == 0), stop=False)
                    for c2 in range(2 * DC):
                        rhs = xTh[c2][:, sl] if c2 < DC else xTl[c2 - DC][:, sl]
                        nc.tensor.matmul(ps[:], wqk_x[c2][:, msl], rhs,
                                         start=False, stop=(c2 == 2 * DC - 1))
                    # unpack into per-head limb layouts; DVE ops stay
                    # partition-aligned, cross-half duplicates go via ACT.
                    for s in range(2):
                        rsl = slice(64 * s, 64 * s + 64)
                        osl = slice(64 * (1 - s), 64 * (1 - s) + 64)
                        bsub = bqk[m][rsl]
                        if m < MH:          # q part: heads 2m, 2m+1
                            hh = 2 * m + s
                            nc.vector.tensor_scalar(
                                Qh[hh][rsl, sl], ps[rsl, :], bsub, None, ALU.add)
                            nc.vector.scalar_tensor_tensor(
                                Ql[hh][rsl, sl], ps[rsl, :], bsub,
                                Qh[hh][rsl, sl], ALU.add, ALU.subtract)
                            nc.scalar.activation(Qh[hh][osl, sl],
                                                 Qh[hh][rsl, sl], AFT.Copy)
                            nc.scalar.activation(Ql[hh][osl, sl],
                                                 Ql[hh][rsl, sl], AFT.Copy)
                        else:               # k part: heads 2(m-MH), +1
                            hh = 2 * (m - MH) + s
                            if s == 0:
                                klo = kscr.tile([128, nqs], FP16, tag="kscr", name=f"klo{nq}_{m}")
                                nc.vector.tensor_scalar(
                                    K2[hh][0:64, sl], ps[0:64, :], bsub,
                                    None, ALU.add)
                                nc.vector.scalar_tensor_tensor(
                                    klo[0:64, :], ps[0:64, :], bsub,
                                    K2[hh][0:64, sl], ALU.add, ALU.subtract)
                                nc.scalar.activation(K2[hh][64:128, sl],
                                                     klo[0:64, :], AFT.Copy)
                            else:
                                khi = kscr.tile([128, nqs], FP16, tag="kscr", name=f"khi{nq}_{m}")
                                nc.vector.tensor_scalar(
                                    khi[64:128, :], ps[64:128, :], bsub,
                                    None, ALU.add)
                                nc.vector.scalar_tensor_tensor(
                                    K2[hh][64:128, sl], ps[64:128, :], bsub,
                                    khi[64:128, :], ALU.add, ALU.subtract)
                                nc.scalar.activation(K2[hh][0:64, sl],
                                                     khi[64:128, :], AFT.Copy)
                for nt in range(4 * nq, 4 * nq + 4):
                    ps = psBv.tile([128, inner], FP32, tag="psBv", name=f"psBv{nt}")
                    tsl = slice(nt * 128, (nt + 1) * 128)
                    for c in range(DC):
                        nc.tensor.matmul(ps[:], xTh[c][:, tsl], wv_h[c][:],
                                         start=(c == 0), stop=False)
                    nc.tensor.matmul(ps[:], ones_row_f[:], bv_row[:],
                                     start=False, stop=True)
                    nc.vector.tensor_copy(V_sb[nt][:], ps[:])

    # ---------------- phase C: attention ----------------
    order = [(h, qc) for h in range(hc) for qc in range(QC)]
    NS = len(order)                 # 16 stages

    with tc.tile_pool(name="psS", bufs=2, space="PSUM") as psS, \
         tc.tile_pool(name="psZ", bufs=2, space="PSUM") as psZp, \
         tc.tile_pool(name="psCB", bufs=1, space="PSUM") as psCBp, \
         tc.tile_pool(name="psO", bufs=1, space="PSUM") as psOp, \
         tc.tile_pool(name="Epool", bufs=2 * KT, space="SBUF") as Ep, \
         tc.tile_pool(name="Fpool", bufs=6, space="SBUF") as Fp, \
         tc.tile_pool(name="mp", bufs=KT + 2, space="SBUF") as mp, \
         tc.tile_pool(name="zqp", bufs=2) as zqp, \
         tc.tile_pool(name="rbp", bufs=3) as rbp:

        # per-stage runtime state
        E_of = {}      # i -> list of 8 E tiles [128, 1024]
        F_of = {}      # i -> list of 4 folded tiles [128, 1024]
        psZ_of = {}    # i -> psZ tile
        cb_of = {}     # i -> (psCB tile, r_bc tile)
        P_of = {}      # i -> list of 8 P tiles [128, 1024]

        def s_block(i, kt):
            h, qc = order[i]
            qsl = slice(qc * qch, (qc + 1) * qch)
            ps = psS.tile([128, 2 * qch], FP32, tag="S", name=f"S{i}_{kt}")
            for j in range(2):
                kc = 2 * kt + j
                ksl = slice(kc * 128, (kc + 1) * 128)
                out = ps[:, j * qch:(j + 1) * qch]
                lhsT = K2[h][:, ksl]
                nc.tensor.matmul(out, lhsT, Qh[h][:, qsl],
                                 start=True, stop=False)
                nc.tensor.matmul(out, lhsT, Ql[h][:, qsl],
                                 start=False, stop=True)
            Et = Ep.tile([128, 2 * qch], FP32, tag="E", name=f"E{i}_{kt}")
            nc.scalar.activation(Et[:], ps[:], AFT.Exp, scale=scale)
            E_of[i].append(Et)

        def fold(i, p):
            """GPSIMD: F[p] = E[2p] + E[2p+1] (fp32, exact)."""
            E = E_of[i]
            Ft = Fp.tile([128, 2 * qch], FP32, tag="F", name=f"F{i}_{p}")
            nc.gpsimd.tensor_tensor(Ft[:], E[2 * p][:], E[2 * p + 1][:],
                                    ALU.add)
            F_of[i].append(Ft)

        def z_mms(i, half):
            """Four fp32 Z matmuls (one concurrent quad) for stage i."""
            if i == 0 and half == 0:
                psZ_of[i] = psZp.tile([128, qch], FP32, tag="Z", name=f"Z{i}")
            elif half == 0:
                psZ_of[i] = psZp.tile([128, qch], FP32, tag="Z", name=f"Z{i}")
            psZ_t = psZ_of[i]
            F = F_of[i]
            for k in range(4):
                idx = 4 * half + k
                p, j = idx // 2, idx % 2
                g = idx % 4
                nc.tensor.matmul(
                    psZ_t[32 * g:32 * g + 32, :], ones32_f[:],
                    F[p][:, j * qch:(j + 1) * qch],
                    start=(idx < 4), stop=(idx >= 4),
                    tile_position=(0, 32 * g))

        def bhead(i):
            """Z finalize for stage i: zq copy, threshold bcast, recip."""
            zq = zqp.tile([128, qch], FP32, tag="zq", name=f"zq{i}")
            nc.vector.tensor_copy(zq[:], psZ_of[i][:])
            psCB = psCBp.tile([128, qch], FP32, tag="CB", name=f"CB{i}")
            nc.tensor.matmul(psCB[:], cq128[:], zq[:], start=True, stop=True)
            # every row of psCB is 0.01*Z, so a full-width elementwise
            # reciprocal yields the broadcast 1/(0.01 Z) directly.
            r_bc = rbp.tile([128, qch], FP32, tag="rbc", name=f"rbc{i}")
            nc.vector.reciprocal_approx_fast(out=r_bc[:], in_=psCB[:])
            cb_of[i] = (psCB, r_bc)

        def masks(i):
            psCB, _ = cb_of[i]
            cb_b = psCB[:].unsqueeze(1).broadcast_to([128, 2, qch])
            for kt in range(KT):
                Pt = mp.tile([128, 2 * qch], BF16, tag="P", name=f"P{i}_{kt}")
                nc.vector._custom_dve(MASK_OP, out=Pt[:], in0=E_of[i][kt][:],
                                      in1=cb_b)
                P_of[i].append(Pt)

        def pv_pair(ia, ib):
            """Column-paired PV for two stages + output scale."""
            psO = psOp.tile([128, qch], FP32, tag="O", name=f"O{ia}")
            for kc in range(KC):
                kt, j = kc // 2, kc % 2
                jsl = slice(j * qch, (j + 1) * qch)
                for z, i_s in ((0, ia), (1, ib)):
                    h = order[i_s][0]
                    nc.tensor.matmul(psO[64 * z:64 * z + 64, :],
                                     V_sb[kc][:, h * dh:(h + 1) * dh],
                                     P_of[i_s][kt][:, jsl],
                                     start=(kc == 0), stop=(kc == KC - 1))
            for z, i_s in ((0, ia), (1, ib)):
                h, qc = order[i_s]
                mq, rq = h // 2, 64 * (h % 2)
                qsl_h = slice(rq, rq + 64)
                qsl = slice(qc * qch, (qc + 1) * qch)
                nc.scalar.activation(attnB[mq][qsl_h, qsl],
                                     psO[64 * z:64 * z + 64, :], AFT.Copy)
                r_bc = cb_of[i_s][1]
                nc.vector.scalar_tensor_tensor(
                    attnB[mq][qsl_h, qsl], attnB[mq][qsl_h, qsl], 0.01,
                    r_bc[qsl_h, :], ALU.mult, ALU.mult)

        for i in range(NS):
            E_of[i] = []
            F_of[i] = []
            P_of[i] = []
            s_block(i, 0)
            s_block(i, 1)
            if i >= 1:
                z_mms(i - 1, 0)
            s_block(i, 2)
            if i >= 1:
                z_mms(i - 1, 1)
            s_block(i, 3)
            if i >= 1:
                bhead(i - 1)
            s_block(i, 4)
            fold(i, 0)
            s_block(i, 5)
            fold(i, 1)
            if i >= 1:
                masks(i - 1)
            s_block(i, 6)
            fold(i, 2)
            s_block(i, 7)
            fold(i, 3)
            if i >= 2 and (i - 1) % 2 == 1:
                pv_pair(i - 2, i - 1)
                del E_of[i - 2], F_of[i - 2], P_of[i - 2]
        # epilogue for the last stage
        i = NS - 1
        z_mms(i, 0)
        z_mms(i, 1)
        bhead(i)
        masks(i)
        pv_pair(i - 1, i)

    # ---------------- phase E: output projection ----------------
    with tc.tile_pool(name="psE", bufs=4, space="PSUM") as psE, \
         tc.tile_pool(name="ostage", bufs=4) as osp:
        for nt in range(NT):
            ps = psE.tile([128, dim], FP32, tag="psE", name=f"psE{nt}")
            tsl = slice(nt * 128, (nt + 1) * 128)
            for m in range(MH):
                nc.tensor.matmul(ps[:], attnB[m][:, tsl], wout[m][:],
                                 start=(m == 0), stop=(m == MH - 1))
            ot = osp.tile([128, dim], FP32, tag="ostage", name=f"ost{nt}")
            if nt % 2 == 0:
                nc.vector.tensor_copy(ot[:], ps[:])
            else:
                nc.scalar.activation(ot[:], ps[:], AFT.Copy)
            nc.sync.dma_start(io["out"][tsl, :], ot[:])


def build_program(n=2048, dim=512, hc=4, dh=64, qch=512):
    nc = bacc.Bacc(trn_type="TRN2", target_bir_lowering=False, debug=False)
    inner = hc * dh
    io = {}

    def din(name, shape, dt):
        io[name] = nc.dram_tensor(name, shape, dt, kind="ExternalInput").ap()

    din("xh", [dim, n], FP16)
    din("xl", [dim, n], FP16)
    din("wqk_h", [dim, 2 * inner], FP16)
    din("wqk_x", [2 * dim, 2 * inner], FP16)
    din("wv_h", [dim, inner], FP16)
    din("bqk", [2 * inner, 1], FP32)
    din("bv", [1, inner], FP32)
    din("wout_b", [inner, dim], BF16)
    io["out"] = nc.dram_tensor("out", [n, dim], FP32, kind="ExternalOutput").ap()

    with tile.TileContext(nc) as tc:
        with ExitStack() as ctx:
            emit_core_kernel(ctx, tc, io, n=n, dim=dim, hc=hc, dh=dh, qch=qch)
    nc.compile()
    return nc


def make_core_inputs(x_b, Wq, Wk, Wv, bq, bk, bv, Wout_g, n=2048, dim=512,
                     hc=4, dh=64):
    """Host-side prep of one core's input dict (numpy, correct dtypes)."""
    f16 = np.float16
    inner = hc * dh
    xT = np.ascontiguousarray(x_b.T)                    # [dim, n]
    xh = xT.astype(f16)
    xl = (xT - xh.astype(np.float32)).astype(f16)
    wqk = np.concatenate([Wq, Wk], axis=1)              # [dim, 2*inner]
    wqk_hi = wqk.astype(f16)
    wqk_lo = (wqk - wqk_hi.astype(np.float32)).astype(f16)
    wqk_x = np.concatenate([wqk_lo, wqk_hi], axis=0)    # [2*dim, 2*inner]
    return {
        "xh": xh, "xl": xl,
        "wqk_h": wqk_hi, "wqk_x": wqk_x,
        "wv_h": Wv.astype(f16),
        "bqk": np.concatenate([bq, bk]).reshape(2 * inner, 1).astype(np.float32),
        "bv": bv.reshape(1, inner).astype(np.float32),
        "wout_b": Wout_g.astype(ml_dtypes.bfloat16),
    }


@functools.lru_cache(maxsize=1)
def _cached_program():
    return build_program()


def kernel(x, Wqkv, bqkv, Wout, bout):
    x = np.asarray(x, dtype=np.float32)
    Wqkv = np.asarray(Wqkv, dtype=np.float32)
    bqkv = np.asarray(bqkv, dtype=np.float32)
    Wout = np.asarray(Wout, dtype=np.float32)
    bout = np.asarray(bout, dtype=np.float32)

    b, n, dim = x.shape
    H, dh = 8, 64
    inner = H * dh
    hc = 4  # heads per core
    Wq, Wk, Wv = Wqkv[:, :inner], Wqkv[:, inner:2 * inner], Wqkv[:, 2 * inner:]
    bq, bk, bv = bqkv[:inner], bqkv[inner:2 * inner], bqkv[2 * inner:]

    in_maps = []
    for c in range(8):
        bb, g = c // 2, c % 2
        hsl = slice(g * hc * dh, (g + 1) * hc * dh)
        in_maps.append(make_core_inputs(
            x[bb], Wq[:, hsl], Wk[:, hsl], Wv[:, hsl],
            bq[hsl], bk[hsl], bv[hsl], Wout[hsl, :],
            n=n, dim=dim, hc=hc, dh=dh))

    nc = _cached_program()
    res = bass_utils.run_bass_kernel_spmd(nc, in_maps, core_ids=list(range(8)))
    global LAST_RESULTS
    LAST_RESULTS = res
    out = np.empty((b, n, dim), dtype=np.float32)
    for bb in range(b):
        out[bb] = res.results[2 * bb]["out"] + res.results[2 * bb + 1]["out"] \
            + bout
    return out
